# revision 30
# baseline (speedup 1.0000x reference)
"""Bi-Real BasicBlock (binary 3x3 conv + BN(eval) + residual) on 8 TRN2 cores.

Strategy: data-parallel over batch (32 images -> 4 per core). Weights are
binarized on host (sign(W); the per-channel scale is folded into the BN
affine) and replicated to every core. x ships as fp16 (halves the input DMA;
sign() is unaffected down to |x|~3e-8 and the residual add loses <0.003
absolute vs an output scale of ~130). On each core, per image:
  1. DMA x[b] in four row-quarters (16 rows each incl. a 2-row conv halo) so
     binarization can start before the full image lands.
  2. ScalarE computes sign(x) -> fp8/bf16 into the interior of a
     zero-bordered [128, 58*58] padded tile (per-quarter ops).
  3. TensorE computes the 3x3 binary conv as accumulating matmuls over
     Cin=128 partitions into 8 PSUM banks (7 output rows per bank). Each
     matmul streams a contiguous flat run of the padded tile (7*58-2 = 404
     cols; the 2 junk outputs per row boundary are skipped at evacuation).
     In fp8 mode the 9 taps run as 4 DoubleRow pair-matmuls (2 MACs/cycle,
     pairing consecutive taps in flat-offset order) plus 1 normal matmul.
     Weights stay loaded across the 4 chunks of each half-image.
  4. VectorE evacuates PSUM with the BN scale and residual fused in one op:
     out = psum * alpha + x  (scalar_tensor_tensor). A nonzero BN shift is
     pre-added into the residual tile on device (shift is zero for eval-mode
     BN with zero running_mean/beta, so that variant is compiled on demand).
  5. Results DMA out per quarter-image.
"""

import os
import sys

for _p in ("/opt/trn_rl_repo", "/root/.axon_site/_ro/trn_rl_repo"):
    if os.path.isdir(_p) and _p not in sys.path:
        sys.path.append(_p)

import numpy as np
import ml_dtypes

B, CIN, H, W_, COUT = 32, 128, 56, 56, 128
HW = H * W_              # 3136
PH, PW = H + 2, W_ + 2   # 58x58 padded
N_CORES = 8
PER = B // N_CORES       # 4 images per core
CH_ROWS = 7              # output rows per PSUM chunk
N_CHUNKS = H // CH_ROWS  # 8
CHUNK = CH_ROWS * W_     # 392
NCOLS = CH_ROWS * PW - 2  # 404 matmul columns (incl. junk at row seams)
QROWS = 14               # quarter height (output rows)
QHALO = 2                # extra x rows DMA'd per quarter for the conv halo
BN_EPS = 1e-5

MODE = os.environ.get("BIREAL_MODE", "fp8")  # "fp8" (DoubleRow) or "bf16"

# fp8 tap pairing: 9 taps in flat-offset order (kh*58+kw) are grouped into
# 4 DoubleRow pairs + 1 single. Pairs may span kernel rows: the rhs pair
# step is just the flat-offset difference.
PAIRS = [((0, 0), (0, 1)), ((0, 2), (1, 0)), ((1, 1), (1, 2)), ((2, 0), (2, 1))]
SINGLE = (2, 2)

_COMPILED = {}


def _build(has_shift):
    import concourse.bass as bass
    import concourse.tile as tile
    from concourse import bacc, mybir

    f32 = mybir.dt.float32
    f16 = mybir.dt.float16
    act_dt = mybir.dt.float8e4 if MODE == "fp8" else mybir.dt.bfloat16
    AF = mybir.ActivationFunctionType
    ALU = mybir.AluOpType

    nc = bacc.Bacc(None, target_bir_lowering=False, debug=False)

    x_d = nc.dram_tensor("x", [PER, CIN, HW], f16, kind="ExternalInput")
    if MODE == "fp8":
        wtp_d = nc.dram_tensor("wtp", [CIN, 4, 2, COUT], act_dt, kind="ExternalInput")
        wts_d = nc.dram_tensor("wts", [CIN, COUT], act_dt, kind="ExternalInput")
    else:
        wt_d = nc.dram_tensor("wt", [CIN, 9, COUT], act_dt, kind="ExternalInput")
    al_d = nc.dram_tensor("alpha", [COUT, 1], f32, kind="ExternalInput")
    sh_d = nc.dram_tensor("shift", [COUT, 1], f32, kind="ExternalInput")
    y_d = nc.dram_tensor("y", [PER, COUT, HW], f32, kind="ExternalOutput")

    with tile.TileContext(nc) as tc:
        with (
            tc.tile_pool(name="consts", bufs=1) as consts,
            tc.tile_pool(name="xin", bufs=16) as xin,
            tc.tile_pool(name="acts", bufs=4) as acts,
            tc.tile_pool(name="outs", bufs=3) as outs,
            tc.tile_pool(name="psum", bufs=8, space=bass.MemorySpace.PSUM) as psum,
        ):
            # consts go on the ScalarE HWDGE ring so they never delay the
            # first x-quarter DMA on the sync ring.
            if MODE == "fp8":
                wp_sb = consts.tile([CIN, 4, 2, COUT], act_dt)
                nc.scalar.dma_start(wp_sb[:], wtp_d[:])
                ws_sb = consts.tile([CIN, COUT], act_dt)
                nc.scalar.dma_start(ws_sb[:], wts_d[:])
            else:
                w_sb = consts.tile([CIN, 9, COUT], act_dt)
                nc.scalar.dma_start(w_sb[:], wt_d[:])
            al_sb = consts.tile([COUT, 1], f32)
            nc.scalar.dma_start(al_sb[:], al_d[:])
            sh_sb = consts.tile([COUT, 1], f32)
            nc.scalar.dma_start(sh_sb[:], sh_d[:])

            # HAM warmup: ~3.9us of near-100%-duty matmuls on a zeroed tile
            # while the first input DMA is in flight, so the PE clock gate
            # (1.2 -> 2.4 GHz) is already released when real matmuls start.
            warm = consts.tile([CIN, 128], act_dt)
            nc.vector.memset(warm[:], 0.0)
            wps = psum.tile([64, 128], f32, tag="ps", name="warmps")
            for i in range(36):
                nc.tensor.matmul(
                    wps[:], warm[:, :64], warm[:],
                    start=(i == 0), stop=(i == 35),
                )

            for b in range(PER):
                # --- input quarters (16 rows incl. 2-row halo; last = 14) ---
                xq = []
                for q in range(4):
                    rows = min(QROWS + QHALO, H - QROWS * q)
                    t = xin.tile([CIN, (QROWS + QHALO) * W_], f16, tag="xq")
                    if b == 0 and q < 2:
                        # split the first transfers in two so each sign op
                        # (chunk c needs only ~9 rows) starts as soon as the
                        # first small piece lands
                        q0 = QROWS * q * W_
                        h0 = 9 * W_
                        nc.sync.dma_start(t[:, :h0], x_d[b, :, q0 : q0 + h0])
                        nc.sync.dma_start(
                            t[:, h0 : rows * W_],
                            x_d[b, :, q0 + h0 : q0 + rows * W_],
                        )
                    else:
                        # image 0's later quarters ride the scalar ring so
                        # the startup input stream uses two DMA queues
                        eng = nc.scalar if (b == 0 and q >= 2) else nc.sync
                        eng.dma_start(
                            t[:, : rows * W_],
                            x_d[b, :, QROWS * q * W_ : (QROWS * q + rows) * W_],
                        )
                    xq.append((t, rows))

                # --- padded sign tile ---
                a_sb = acts.tile([CIN, PH * PW], act_dt)
                av = a_sb[:].rearrange("p (h w) -> p h w", w=PW)
                nc.vector.memset(av[:, 0, :], 0.0)
                nc.vector.memset(av[:, PH - 1, :], 0.0)
                nc.vector.memset(av[:, 1 : PH - 1, 0:1], 0.0)
                nc.vector.memset(av[:, 1 : PH - 1, PW - 1 : PW], 0.0)
                for q in range(4):
                    t, rows = xq[q]
                    r0 = 1 + QROWS * q
                    if b == 0 and q < 2:
                        # two sign ops matching the split first DMAs
                        nc.scalar.activation(
                            av[:, r0 : r0 + 9, 1 : 1 + W_],
                            t[:, : 9 * W_].rearrange("p (h w) -> p h w", w=W_),
                            AF.Sign,
                        )
                        nc.scalar.activation(
                            av[:, r0 + 9 : r0 + rows, 1 : 1 + W_],
                            t[:, 9 * W_ : rows * W_].rearrange(
                                "p (h w) -> p h w", w=W_
                            ),
                            AF.Sign,
                        )
                    else:
                        nc.scalar.activation(
                            av[:, r0 : r0 + rows, 1 : 1 + W_],
                            t[:, : rows * W_].rearrange("p (h w) -> p h w", w=W_),
                            AF.Sign,
                        )
                    if has_shift:
                        # fold the BN shift into the residual tile in place
                        # (only the 14 quarter-body rows are used as residual)
                        nc.vector.tensor_scalar(
                            t[:, : QROWS * W_], t[:, : QROWS * W_],
                            sh_sb[:], None, op0=ALU.add,
                        )

                o_sb = outs.tile([COUT, HW], f32)
                base = a_sb[:]
                for half in range(2):
                    pss = [
                        psum.tile([COUT, NCOLS], f32, tag="ps", name="ps")
                        for _ in range(4)
                    ]
                    if MODE == "fp8":
                        taps = [("p", i) for i in range(len(PAIRS))] + [("s", 0)]
                    else:
                        taps = [("b", t) for t in range(9)]
                    # tap-outer / chunk-inner: weights stay loaded across
                    # the 4 chunks of this half-image. The very first half-
                    # image runs chunk-outer instead so the PE can start on
                    # the first signed quarter before the second one lands.
                    cis = list(range(4))
                    if b == PER - 1 and half == 1:
                        # kernel tail: run chunk-outer in reverse so each
                        # bank completes (and evacuates) as early as possible
                        # and only the last chunk's work trails the final MM
                        cis = cis[::-1]
                    if (b == 0 and half == 0) or (b == PER - 1 and half == 1):
                        order = [
                            (ti, ci)
                            for ci in cis
                            for ti in range(len(taps))
                        ]
                    else:
                        order = [
                            (ti, ci)
                            for ti in range(len(taps))
                            for ci in cis
                        ]
                    for ti, ci in order:
                        kind, k = taps[ti]
                        start = ti == 0
                        stop = ti == len(taps) - 1
                        if True:
                            c = 4 * half + ci
                            cbase = base.offset + CH_ROWS * c * PW
                            if kind == "p":
                                (ka, kb) = PAIRS[k]
                                offa = ka[0] * PW + ka[1]
                                step = kb[0] * PW + kb[1] - offa
                                rhs = bass.AP(
                                    tensor=base.tensor,
                                    offset=cbase + offa,
                                    ap=[base.ap[0], [step, 2], [1, NCOLS]],
                                )
                                nc.tensor.matmul(
                                    pss[ci][:],
                                    wp_sb[:, k, :, :],
                                    rhs,
                                    start=start,
                                    stop=stop,
                                    perf_mode=mybir.MatmulPerfMode.DoubleRow,
                                )
                            else:
                                if kind == "s":
                                    kh, kw = SINGLE
                                    lhsT = ws_sb[:]
                                else:
                                    kh, kw = divmod(k, 3)
                                    lhsT = w_sb[:, k, :]
                                rhs = bass.AP(
                                    tensor=base.tensor,
                                    offset=cbase + kh * PW + kw,
                                    ap=[base.ap[0], [1, NCOLS]],
                                )
                                nc.tensor.matmul(
                                    pss[ci][:], lhsT, rhs, start=start, stop=stop
                                )
                    # evacuate on VectorE with BN scale + residual fused:
                    # out = psum * alpha + x(+shift)   (junk cols skipped)
                    for ci in cis:
                        c = 4 * half + ci
                        ps = pss[ci][:]
                        if b == PER - 1 and half == 1 and ci == cis[-1]:
                            # very last chunk: evacuate in two pieces so the
                            # final output DMA can start after the first one
                            pieces = [(0, 4), (4, 3)]
                        else:
                            pieces = [(0, CH_ROWS)]
                        for pr0, prows in pieces:
                            src = bass.AP(
                                tensor=ps.tensor,
                                offset=ps.offset + pr0 * PW,
                                ap=[ps.ap[0], [PW, prows], [1, W_]],
                            )
                            csl = slice(
                                CHUNK * c + pr0 * W_,
                                CHUNK * c + (pr0 + prows) * W_,
                            )
                            dst = o_sb[:, csl].rearrange("p (h w) -> p h w", w=W_)
                            rsl = slice(
                                CHUNK * (c % 2) + pr0 * W_,
                                CHUNK * (c % 2) + (pr0 + prows) * W_,
                            )
                            res = xq[c // 2][0][:, rsl].rearrange(
                                "p (h w) -> p h w", w=W_
                            )
                            nc.vector.scalar_tensor_tensor(
                                dst, src, al_sb[:], res, op0=ALU.mult, op1=ALU.add
                            )
                    # DMA this half's two quarters out
                    if b == PER - 1 and half == 1:
                        # kernel tail: issue the last quarter as two
                        # chunk-sized DMAs on the idle scalar ring so the
                        # first starts flushing before the last evac lands
                        for c in (7, 6):
                            sl = slice(CHUNK * c, CHUNK * (c + 1))
                            nc.scalar.dma_start(y_d[b, :, sl], o_sb[:, sl])
                        sl = slice(CHUNK * 5, CHUNK * 6)
                        nc.gpsimd.dma_start(y_d[b, :, sl], o_sb[:, sl])
                        sl = slice(CHUNK * 4, CHUNK * 4 + 4 * W_)
                        nc.gpsimd.dma_start(y_d[b, :, sl], o_sb[:, sl])
                        sl = slice(CHUNK * 4 + 4 * W_, CHUNK * 5)
                        nc.scalar.dma_start(y_d[b, :, sl], o_sb[:, sl])
                    else:
                        for q in (2 * half, 2 * half + 1):
                            sl = slice(QROWS * W_ * q, QROWS * W_ * (q + 1))
                            nc.gpsimd.dma_start(y_d[b, :, sl], o_sb[:, sl])

    nc.compile()
    return nc


def _get_compiled(has_shift):
    key = (MODE, bool(has_shift))
    if key not in _COMPILED:
        _COMPILED[key] = _build(has_shift)
    return _COMPILED[key]


def _prep_in_maps(x, W, gamma, beta, running_mean, running_var):
    x = np.asarray(x, dtype=np.float32)
    W = np.asarray(W, dtype=np.float32)
    gamma = np.asarray(gamma, dtype=np.float32)
    beta = np.asarray(beta, dtype=np.float32)
    running_mean = np.asarray(running_mean, dtype=np.float32)
    running_var = np.asarray(running_var, dtype=np.float32)

    scale = np.abs(W).mean(axis=(1, 2, 3))              # [Cout]
    inv = gamma / np.sqrt(running_var + BN_EPS)          # [Cout]
    alpha = (scale * inv).astype(np.float32)[:, None]    # [Cout, 1]
    shift = (beta - running_mean * inv).astype(np.float32)[:, None]

    # wsign[i, kh, kw, o] = sign(W[o, i, kh, kw])
    wsign = np.sign(W).transpose(1, 2, 3, 0)
    act_np = ml_dtypes.float8_e4m3 if MODE == "fp8" else ml_dtypes.bfloat16

    xr = np.ascontiguousarray(x.reshape(B, CIN, HW)).astype(np.float16)
    common = {"alpha": alpha, "shift": shift}
    if MODE == "fp8":
        wtp = np.stack(
            [
                np.stack([wsign[:, ka[0], ka[1], :], wsign[:, kb[0], kb[1], :]], axis=1)
                for (ka, kb) in PAIRS
            ],
            axis=1,
        )  # [CIN, 4, 2, COUT]
        common["wtp"] = np.ascontiguousarray(wtp).astype(act_np)
        common["wts"] = np.ascontiguousarray(wsign[:, SINGLE[0], SINGLE[1], :]).astype(
            act_np
        )
    else:
        common["wt"] = np.ascontiguousarray(wsign.reshape(CIN, 9, COUT)).astype(act_np)

    has_shift = bool(np.any(shift != 0.0))
    in_maps = []
    for c in range(N_CORES):
        in_maps.append({"x": xr[c * PER : (c + 1) * PER], **common})
    return in_maps, has_shift


def _install_axon_trace_support():
    """Register the NTFF profiling hook that this image's antenv lacks.

    Only used by kernel_timed(); the plain kernel() path never traces.
    """
    import types

    if "antenv.axon_hooks" not in sys.modules:
        mod = types.ModuleType("antenv.axon_hooks")
        mod._hook = None

        def set_axon_ntff_profile_hook(h):
            mod._hook = h

        def get_axon_ntff_profile_hook():
            return mod._hook

        mod.set_axon_ntff_profile_hook = set_axon_ntff_profile_hook
        mod.get_axon_ntff_profile_hook = get_axon_ntff_profile_hook
        sys.modules["antenv.axon_hooks"] = mod
        import antenv

        antenv.axon_hooks = mod
    hooks = sys.modules["antenv.axon_hooks"]
    if hooks.get_axon_ntff_profile_hook() is None:
        from trn_agent_boot.trn_boot import _ntff_profile_via_ctypes

        hooks.set_axon_ntff_profile_hook(
            _ntff_profile_via_ctypes("/opt/axon/libaxon_pjrt.so")
        )
    # No S3 bucket in this sandbox; keep artifacts local.
    from concourse import bass_utils

    bass_utils.upload_artifacts = lambda tmpdir: tmpdir


def _run(in_maps, has_shift, trace=False, tmpdir=None):
    from concourse.bass_utils import run_bass_kernel_spmd

    if trace:
        _install_axon_trace_support()
    nc = _get_compiled(has_shift)
    res = run_bass_kernel_spmd(
        nc, in_maps, list(range(N_CORES)), trace=trace, tmpdir=tmpdir
    )
    y = np.concatenate([res.results[c]["y"] for c in range(N_CORES)], axis=0)
    return y.reshape(B, COUT, H, W_).astype(np.float32), res


def kernel(x, W, gamma, beta, running_mean, running_var):
    in_maps, has_shift = _prep_in_maps(x, W, gamma, beta, running_mean, running_var)
    last_err = None
    for _attempt in range(3):
        try:
            y, _ = _run(in_maps, has_shift, trace=False)
            return y
        except Exception as e:  # transient NRT device errors recover on retry
            last_err = e
    raise last_err


def kernel_timed(x, W, gamma, beta, running_mean, running_var, tmpdir=None):
    """Like kernel() but also returns the profiled HW execution time in ns."""
    in_maps, has_shift = _prep_in_maps(x, W, gamma, beta, running_mean, running_var)
    y, res = _run(in_maps, has_shift, trace=True, tmpdir=tmpdir)
    return y, res
